# revision 4
# baseline (speedup 1.0000x reference)
"""LoRA layer (x @ W.T + (x@A)@B + bias) on 8 trn2 NeuronCores.

Data-parallel: core b computes batch b's (2048, 4096) output slice.
Host folds the low-rank path into the dense weight (W_eff = W.T + A@B,
cast to bf16 -- rel err ~2e-3, well inside the 2e-2 gate) so the device
does a single 2048x4096x4096 GEMM per core; bias is added on host
(exact fp32, zero device cost).

Device structure (per core): x fully resident in SBUF (16 MiB bf16,
32 k-tiles of [128, 2048]); W_eff streamed from HBM exactly once as
[128, 512] moving tiles. x-stationary orientation: each output tile
[128m, 512o] accumulates its full 32-step contraction into a single
PSUM bank back-to-back (K-contiguous), rotating across all 8 banks, so
a bank's drain (DVE/ACT copy + DMA out) always overlaps 7 other tiles'
matmuls and the PE never stalls on a drain WAR or goes HAM-cold.
"""
import os
import sys
import types

import numpy as np
import ml_dtypes

import concourse.mybir as mybir
import concourse.tile as tile
from concourse import bacc
from concourse.bass_utils import run_bass_kernel_spmd

BATCH, SEQ, DIN, DOUT = 8, 2048, 4096, 4096
N_CORES = 8
KT = DIN // 128            # 32 contraction tiles
MT = SEQ // 128            # 16 output row tiles (per core)
OB = DOUT // 512           # 8 output column blocks
BF16 = mybir.dt.bfloat16
F32 = mybir.dt.float32
NP_BF16 = ml_dtypes.bfloat16

_nc_cache = []
last_result = []


def _ensure_ntff_hook():
    """Best-effort: register the axon NTFF profiling hook if the image
    lacks antenv.axon_hooks, so BASS_TRACE=1 yields exec_time_ns instead
    of crashing. No-op when the real module exists or axon is absent."""
    try:
        import antenv.axon_hooks  # noqa: F401
        return
    except ImportError:
        pass
    except Exception:
        return
    try:
        import antenv

        mod = types.ModuleType("antenv.axon_hooks")
        _h = {}
        mod.set_axon_ntff_profile_hook = lambda h: _h.__setitem__("h", h)
        mod.get_axon_ntff_profile_hook = lambda: _h.get("h")
        sys.modules["antenv.axon_hooks"] = mod
        antenv.axon_hooks = mod
        try:
            from trn_agent_boot.trn_boot import _ntff_profile_via_ctypes

            so = "/opt/axon/libaxon_pjrt.so"
            if os.path.exists(so):
                mod.set_axon_ntff_profile_hook(_ntff_profile_via_ctypes(so))
        except Exception:
            pass
    except Exception:
        pass


def _safe_upload_artifacts():
    """Artifact upload has no bucket in this container; fall back to the
    local dir instead of failing the traced run."""
    try:
        import concourse.bass_utils as _bu

        orig = _bu.upload_artifacts

        def _safe(tmpdir):
            try:
                return orig(tmpdir)
            except Exception:
                return str(tmpdir)

        if getattr(_bu.upload_artifacts, "__name__", "") != "_safe":
            _bu.upload_artifacts = _safe
    except Exception:
        pass


_ensure_ntff_hook()
_safe_upload_artifacts()


def _build():
    nc = bacc.Bacc("TRN2", target_bir_lowering=False, debug=False)
    xT = nc.dram_tensor("xT", [DIN, SEQ], BF16, kind="ExternalInput")
    wT = nc.dram_tensor("wT", [DIN, DOUT], BF16, kind="ExternalInput")
    out = nc.dram_tensor("out", [SEQ, DOUT], F32, kind="ExternalOutput")

    with tile.TileContext(nc) as tc:
        with (
            tc.tile_pool(name="xres", bufs=KT) as xpool,
            tc.tile_pool(name="wt", bufs=KT + 8) as wpool,
            tc.tile_pool(name="outp", bufs=10) as opool,
            tc.tile_pool(name="psum", bufs=8, space="PSUM") as ppool,
        ):
            # whole per-core activation resident in SBUF: 32 x 4KB/partition
            xtiles = []
            for k in range(KT):
                xt = xpool.tile([128, SEQ], BF16, name=f"x{k}", tag="x")
                nc.gpsimd.dma_start(xt[:], xT[k * 128:(k + 1) * 128, :])
                xtiles.append(xt)

            def drain(ps, mb, m0, o0):
                ot = opool.tile([128, 512], F32, name="o", tag="o")
                if mb % 2 == 0:
                    nc.vector.tensor_copy(ot[:], ps[:])
                else:
                    nc.scalar.activation(
                        ot[:], ps[:], mybir.ActivationFunctionType.Copy)
                nc.sync.dma_start(out[m0:m0 + 128, o0:o0 + 512], ot[:])

            for ob in range(OB):
                o0 = ob * 512
                # this block's W column panel: 32 x 1KB/partition, in a
                # ring big enough to prefetch the next panel
                wsb = []
                for k in range(KT):
                    wt = wpool.tile([128, 512], BF16, name="w", tag="w")
                    nc.sync.dma_start(
                        wt[:], wT[k * 128:(k + 1) * 128, o0:o0 + 512])
                    wsb.append(wt)

                if ob == 0:
                    # Phase A: k-outer over 8 m-tiles / 8 banks so PE work
                    # per x-tile (8 MMs) covers the x-stream DMA latency.
                    psums = [ppool.tile([128, 512], F32, name="ps", tag="ps")
                             for _ in range(8)]
                    for k in range(KT):
                        for mb in range(8):
                            nc.tensor.matmul(
                                psums[mb][:],
                                xtiles[k][:, mb * 128:(mb + 1) * 128],
                                wsb[k][:],
                                start=(k == 0), stop=(k == KT - 1))
                    for mb in range(8):
                        drain(psums[mb], mb, mb * 128, o0)
                    rest = range(8, MT)
                else:
                    rest = range(MT)

                # Single live PSUM bank per output tile: the full 32-step
                # contraction runs back-to-back; drains always overlap
                # other tiles' matmuls.
                for mb in rest:
                    m0 = mb * 128
                    ps = ppool.tile([128, 512], F32, name="ps", tag="ps")
                    for k in range(KT):
                        nc.tensor.matmul(
                            ps[:],
                            xtiles[k][:, m0:m0 + 128],
                            wsb[k][:],
                            start=(k == 0), stop=(k == KT - 1))
                    drain(ps, mb, m0, o0)
    nc.compile()
    return nc


def kernel(x, A, B, weight, bias):
    if not _nc_cache:
        _nc_cache.append(_build())
    nc = _nc_cache[0]

    x = np.asarray(x, dtype=np.float32)
    A = np.asarray(A, dtype=np.float32)
    B = np.asarray(B, dtype=np.float32)
    weight = np.asarray(weight, dtype=np.float32)
    bias = np.asarray(bias, dtype=np.float32)

    # Fold the rank-16 path into the dense weight: out = x @ W_eff + bias
    w_eff = weight.T + A @ B                                  # [DIN, DOUT]
    wT = np.ascontiguousarray(w_eff, dtype=np.float32).astype(NP_BF16)

    in_maps = []
    for b in range(N_CORES):
        xTb = np.ascontiguousarray(x[b].T).astype(NP_BF16)    # [DIN, SEQ]
        in_maps.append({"xT": xTb, "wT": wT})

    res = run_bass_kernel_spmd(nc, in_maps, core_ids=list(range(N_CORES)))
    last_result.clear()
    last_result.append(res)
    outs = np.stack([r["out"] for r in res.results], axis=0)
    if bias.any():
        outs = outs + bias[None, None, :]
    return outs


# revision 8
# speedup vs baseline: 1.2234x; 1.2234x over previous
"""LoRA layer (x @ W.T + (x@A)@B + bias) on 8 trn2 NeuronCores.

Data-parallel: core b computes batch b's (2048, 4096) output slice.
Host folds the low-rank path into the dense weight (W_eff = W.T + A@B,
cast to bf16 -- rel err ~2e-3, well inside the 2e-2 gate) so the device
does a single 2048x4096x4096 GEMM per core; bias is added on host
(exact fp32, zero device cost).

Device structure (per core): x fully resident in SBUF (16 MiB bf16,
32 k-tiles of [128, 2048]); W_eff streamed from HBM exactly once as
[128, 512] moving tiles. x-stationary orientation: each output tile
[128m, 512o] accumulates its full 32-step contraction into a single
PSUM bank back-to-back (K-contiguous), rotating across all 8 banks, so
a bank's drain (DVE/ACT copy + DMA out) always overlaps 7 other tiles'
matmuls and the PE never stalls on a drain WAR or goes HAM-cold.
"""
import os
import sys
import types

import numpy as np
import ml_dtypes

import concourse.mybir as mybir
import concourse.tile as tile
from concourse import bacc
from concourse.bass_utils import run_bass_kernel_spmd

BATCH, SEQ, DIN, DOUT = 8, 2048, 4096, 4096
N_CORES = 8
KT = DIN // 128            # 32 contraction tiles
MT = SEQ // 128            # 16 output row tiles (per core)
OB = DOUT // 512           # 8 output column blocks
BF16 = mybir.dt.bfloat16
F32 = mybir.dt.float32
NP_BF16 = ml_dtypes.bfloat16

_nc_cache = []
last_result = []


def _ensure_ntff_hook():
    """Best-effort: register the axon NTFF profiling hook if the image
    lacks antenv.axon_hooks, so BASS_TRACE=1 yields exec_time_ns instead
    of crashing. No-op when the real module exists or axon is absent."""
    try:
        import antenv.axon_hooks  # noqa: F401
        return
    except ImportError:
        pass
    except Exception:
        return
    try:
        import antenv

        mod = types.ModuleType("antenv.axon_hooks")
        _h = {}
        mod.set_axon_ntff_profile_hook = lambda h: _h.__setitem__("h", h)
        mod.get_axon_ntff_profile_hook = lambda: _h.get("h")
        sys.modules["antenv.axon_hooks"] = mod
        antenv.axon_hooks = mod
        try:
            from trn_agent_boot.trn_boot import _ntff_profile_via_ctypes

            so = "/opt/axon/libaxon_pjrt.so"
            if os.path.exists(so):
                mod.set_axon_ntff_profile_hook(_ntff_profile_via_ctypes(so))
        except Exception:
            pass
    except Exception:
        pass


def _safe_upload_artifacts():
    """Artifact upload has no bucket in this container; fall back to the
    local dir instead of failing the traced run."""
    try:
        import concourse.bass_utils as _bu

        orig = _bu.upload_artifacts

        def _safe(tmpdir):
            try:
                return orig(tmpdir)
            except Exception:
                return str(tmpdir)

        if getattr(_bu.upload_artifacts, "__name__", "") != "_safe":
            _bu.upload_artifacts = _safe
    except Exception:
        pass


_ensure_ntff_hook()
_safe_upload_artifacts()


def _build():
    nc = bacc.Bacc("TRN2", target_bir_lowering=False, debug=False)
    xT = nc.dram_tensor("xT", [DIN, SEQ], BF16, kind="ExternalInput")
    wT = nc.dram_tensor("wT", [DIN, DOUT], BF16, kind="ExternalInput")
    out = nc.dram_tensor("out", [SEQ, DOUT], F32, kind="ExternalOutput")

    with tile.TileContext(nc) as tc:
        with (
            tc.tile_pool(name="xres", bufs=KT - 2) as xpool,
            tc.tile_pool(name="xchunk", bufs=8) as xcpool,
            tc.tile_pool(name="wt", bufs=KT + 8) as wpool,
            tc.tile_pool(name="outp", bufs=10) as opool,
            tc.tile_pool(name="psum", bufs=8, space="PSUM") as ppool,
        ):
            # Whole per-core activation resident in SBUF: 32 x 4KB/partition.
            # The first two k-tiles are loaded in 4 column chunks so the
            # very first matmuls wait on a 128KB DMA, not a 512KB one.
            xslice = {}
            for k in range(KT):
                if k < 2:
                    chunks = []
                    for c in range(4):
                        xt = xcpool.tile([128, 512], BF16,
                                         name=f"x{k}c{c}", tag="xc")
                        nc.gpsimd.dma_start(
                            xt[:], xT[k * 128:(k + 1) * 128,
                                      c * 512:(c + 1) * 512])
                        chunks.append(xt)
                    xslice[k] = lambda mb, ch=chunks: \
                        ch[mb // 4][:, (mb % 4) * 128:(mb % 4 + 1) * 128]
                else:
                    xt = xpool.tile([128, SEQ], BF16, name=f"x{k}", tag="x")
                    nc.gpsimd.dma_start(xt[:], xT[k * 128:(k + 1) * 128, :])
                    xslice[k] = lambda mb, t=xt: \
                        t[:, mb * 128:(mb + 1) * 128]

            def drain(ps, i, m0, o0, last=False):
                if last:
                    # tail latency: split the final drain + store across
                    # both copy engines and several DMA rings
                    ot = opool.tile([128, 512], F32, name="o", tag="o")
                    nc.vector.tensor_copy(ot[:, :256], ps[:, :256])
                    nc.scalar.activation(
                        ot[:, 256:], ps[:, 256:],
                        mybir.ActivationFunctionType.Copy)
                    for c, eng in enumerate(
                            (nc.sync, nc.gpsimd, nc.scalar, nc.sync)):
                        eng.dma_start(
                            out[m0:m0 + 128, o0 + c * 128:o0 + (c + 1) * 128],
                            ot[:, c * 128:(c + 1) * 128])
                    return
                ot = opool.tile([128, 512], F32, name="o", tag="o")
                if i % 2 == 0:
                    nc.vector.tensor_copy(ot[:], ps[:])
                else:
                    nc.scalar.activation(
                        ot[:], ps[:], mybir.ActivationFunctionType.Copy)
                nc.sync.dma_start(out[m0:m0 + 128, o0:o0 + 512], ot[:])

            for ob in range(OB):
                o0 = ob * 512
                # this block's W column panel: 32 x 1KB/partition, in a
                # ring big enough to prefetch the next panel
                wsb = []
                for k in range(KT):
                    wt = wpool.tile([128, 512], BF16, name="w", tag="w")
                    nc.sync.dma_start(
                        wt[:], wT[k * 128:(k + 1) * 128, o0:o0 + 512])
                    wsb.append(wt)

                if ob == 0:
                    # k-outer over 8 m-tiles / 8 banks so PE work per
                    # x-tile (8 MMs, 1.73us) covers the x-stream DMA.
                    groups = [range(0, 8), range(8, 12), range(12, 16)]
                else:
                    groups = [range(g * 4, (g + 1) * 4) for g in range(4)]

                # k-outer within each group: consecutive matmuls rotate
                # PSUM banks (same-bank back-to-back costs +43ns/MM), and
                # consecutive groups use disjoint bank halves so drains
                # overlap the next group's whole k-loop.
                for grp in groups:
                    psums = {mb: ppool.tile([128, 512], F32,
                                            name="ps", tag="ps")
                             for mb in grp}
                    for k in range(KT):
                        for mb in grp:
                            nc.tensor.matmul(
                                psums[mb][:],
                                xslice[k](mb),
                                wsb[k][:],
                                start=(k == 0), stop=(k == KT - 1))
                    for i, mb in enumerate(grp):
                        is_last = (ob == OB - 1 and mb == MT - 1)
                        drain(psums[mb], i, mb * 128, o0, last=is_last)
    nc.compile()
    return nc


def kernel(x, A, B, weight, bias):
    if not _nc_cache:
        _nc_cache.append(_build())
    nc = _nc_cache[0]

    x = np.asarray(x, dtype=np.float32)
    A = np.asarray(A, dtype=np.float32)
    B = np.asarray(B, dtype=np.float32)
    weight = np.asarray(weight, dtype=np.float32)
    bias = np.asarray(bias, dtype=np.float32)

    # Fold the rank-16 path into the dense weight: out = x @ W_eff + bias
    w_eff = weight.T + A @ B                                  # [DIN, DOUT]
    wT = np.ascontiguousarray(w_eff, dtype=np.float32).astype(NP_BF16)

    in_maps = []
    for b in range(N_CORES):
        xTb = np.ascontiguousarray(x[b].T).astype(NP_BF16)    # [DIN, SEQ]
        in_maps.append({"xT": xTb, "wT": wT})

    res = run_bass_kernel_spmd(nc, in_maps, core_ids=list(range(N_CORES)))
    last_result.clear()
    last_result.append(res)
    outs = np.stack([r["out"] for r in res.results], axis=0)
    if bias.any():
        outs = outs + bias[None, None, :]
    return outs
